# revision 1
# baseline (speedup 1.0000x reference)
"""Trainium2 Bass kernel for a two-window sparse causal self-attention block.

Model (B=2, T=2048, C=1024):
  - 8 "short" heads: d_qk=32,  window 256
  - 8 "long"  heads: d_qk=128, window 1024
  - value/output head dim 64, output projection C x C.

Sharding (8 cores): data-parallel over batch (2) x head-parallel over head
groups (4). Core c = 4*b + g handles batch b and heads {2g, 2g+1} of both the
short and long sets. Each core computes its 4 heads' attention plus the
corresponding 256 rows of Wproj, producing a partial [T, C] output; the host
sums the 4 partials per batch element.

Device-side design notes:
  - float32r matmuls everywhere: full PE rate (1 cycle/row at N>=256) vs 2
    cycles/row for fp32, ~1.5e-4 matmul relative error.
  - everything is computed in "transposed" orientation so no on-device
    transposes are needed: host passes xT [C, T]; projections give qT/kT
    [d, T] and v [T, dv]; scores sT[k, q] = kT.T @ qT; yT[dv, q] = v_aug.T @
    pT with a ones column in v so row 64 of yT accumulates softmax sums.
  - queries processed in groups of 512 (4 blocks) so score/AV matmuls run at
    N=512; the causal band mask is applied multiplicatively on exp(scores)
    using 512-wide sliding windows into a host-precomputed [128, W+896] band
    image.
  - exp skips the max-subtraction: inputs are well-scaled (|scores| < ~10).
  - normalization: reciprocal of the sums row, broadcast across partitions
    via a rank-1 matmul against a ones row, multiply into the yT tiles.
"""

import math

import numpy as np

import concourse.bass as bass
import concourse.mybir as mybir
import concourse.tile as tile
from concourse.bass_utils import run_bass_kernel_spmd

F32 = mybir.dt.float32
F32R = mybir.dt.float32r

B, T, C = 2, 2048, 1024
HS, DS = 8, 32
HL, DL = 8, 128
HD = 64
WIN_S, WIN_L = 256, 1024
NT = T // 128    # 16 t-blocks
NCB = C // 128   # 8 c-blocks
NG = T // 512    # 4 query groups
VW = HD + 1      # v columns + ones column for softmax sums
N_CORES = 8


def _split_waits(nc: bass.Bass) -> int:
    """Walrus in this env accepts at most 1 sync wait per instruction.
    Hoist extra waits onto same-engine InstNoOp instructions placed just
    before the owning instruction (same-engine program order preserves the
    blocking semantics)."""
    import bass_rust

    n_added = 0
    for f in nc.m.functions:
        for bb in f.blocks:
            insts = bb.instructions
            if not any(inst.sync_info and len(inst.sync_info.on_wait) > 1
                       for inst in insts):
                continue
            new = []
            for inst in insts:
                si = inst.sync_info
                waits = list(si.on_wait) if si else []
                if len(waits) > 1:
                    for i, w in enumerate(waits[:-1]):
                        nop = mybir.InstNoOp(
                            name=f"{inst.name}_hw{i}",
                            sync_info=bass_rust.SyncInfo(on_wait=[w], on_update=[]),
                            bass_nofuse=True,
                            engine=inst.engine,
                        )
                        new.append(nop)
                        n_added += 1
                    inst.sync_info = bass_rust.SyncInfo(
                        on_wait=waits[-1:], on_update=list(si.on_update))
                new.append(inst)
            bb.instructions = new
    return n_added


def _patch_tile_drain():
    """This walrus build rejects >1 sync wait on the TileContext tail drain
    ("Too many sync wait commands"). Re-emit the drain's waits as individual
    wait_ge instructions on the sync engine."""
    import bass_rust
    from concourse.tile import ScopedClock, TileContext

    def _drain_and_barrier(self, tick_clock, wait_clock):
        nc = self.nc
        drain_inst = nc.sync.drain()
        wait_clock.add_sem_waits(
            drain_inst.ins, ScopedClock({None: tick_clock.global_clock})
        )
        si = drain_inst.ins.sync_info
        waits = list(si.on_wait) if si is not None else []
        if len(waits) > 1:
            drain_inst.ins.sync_info = bass_rust.SyncInfo(on_wait=[], on_update=[])
            sems = {h.name: h for h in self.sems.allocated().values()}
            for w in waits:
                nc.sync.wait_ge(sems[w.ant_name], w.wait_value)
        nc.all_engine_barrier()
        popped = nc._tile_sem_poison_stack.pop()
        assert popped is self._sem_poison
        nc.clear_and_free_semaphores(list(self.sems.allocated().values()))
        nc.all_engine_barrier()

    TileContext._drain_and_barrier = _drain_and_barrier


_patch_tile_drain()


def _build_program() -> bass.Bass:
    nc = bass.Bass()

    xt_d = nc.dram_tensor("xt", [C, T], F32, kind="ExternalInput")
    wsqk_d = nc.dram_tensor("wsqk", [C, 128], F32, kind="ExternalInput")
    wql_d = nc.dram_tensor("wql", [C, 256], F32, kind="ExternalInput")
    wkl_d = nc.dram_tensor("wkl", [C, 256], F32, kind="ExternalInput")
    wv_d = nc.dram_tensor("wv", [C, 256], F32, kind="ExternalInput")
    wp_d = nc.dram_tensor("wp", [256, C], F32, kind="ExternalInput")
    bs_d = nc.dram_tensor("band_s", [128, WIN_S + 896], F32, kind="ExternalInput")
    bl_d = nc.dram_tensor("band_l", [128, WIN_L + 896], F32, kind="ExternalInput")
    ones_d = nc.dram_tensor("ones", [128, 64], F32, kind="ExternalInput")
    out_d = nc.dram_tensor("out", [T, C], F32, kind="ExternalOutput")

    scale_s = 1.0 / math.sqrt(DS)
    scale_l = 1.0 / math.sqrt(DL)

    with tile.TileContext(nc) as tc:
        with (
            tc.tile_pool(name="const", bufs=1) as const,
            tc.tile_pool(name="qkp", bufs=1) as qkp,
            tc.tile_pool(name="vp", bufs=1) as vp,
            tc.tile_pool(name="bigps", bufs=2, space="PSUM") as bigps,
            tc.tile_pool(name="yhps", bufs=2, space="PSUM") as yhps,
            tc.tile_pool(name="rbps", bufs=2, space="PSUM") as rbps,
        ):
            # ---- weights (f32r views of the fp32 DRAM data) ----
            wsqk = const.tile([128, NCB, 128], F32R, tag="wsqk", name="wsqk")
            nc.sync.dma_start(wsqk[:], wsqk_d[:, :].bitcast(F32R).rearrange("(cb p) d -> p cb d", p=128))
            wql = const.tile([128, NCB, 256], F32R, tag="wql", name="wql")
            nc.sync.dma_start(wql[:], wql_d[:, :].bitcast(F32R).rearrange("(cb p) d -> p cb d", p=128))
            wkl = const.tile([128, NCB, 256], F32R, tag="wkl", name="wkl")
            nc.sync.dma_start(wkl[:], wkl_d[:, :].bitcast(F32R).rearrange("(cb p) d -> p cb d", p=128))
            wv = const.tile([128, NCB, 256], F32R, tag="wv", name="wv")
            nc.sync.dma_start(wv[:], wv_d[:, :].bitcast(F32R).rearrange("(cb p) d -> p cb d", p=128))

            # ---- projection outputs (persist across both stages) ----
            # short heads: qts/kts [64, T], rows 0-31 head0, 32-63 head1
            # (separate tiles so score matmul lhsT/rhs base partitions align)
            qts = qkp.tile([64, T], F32R, tag="qts", name="qts")
            kts = qkp.tile([64, T], F32R, tag="kts", name="kts")
            qtl = [qkp.tile([128, T], F32R, tag=f"qtl{h}", name=f"qtl{h}") for h in range(2)]
            ktl = [qkp.tile([128, T], F32R, tag=f"ktl{h}", name=f"ktl{h}") for h in range(2)]
            # v tiles per head, [128, NT*VW]; col 64 of each block = 1.0
            vt = [vp.tile([128, NT * VW], F32R, tag=f"vt{i}", name=f"vt{i}") for i in range(4)]

            # ================= stage A: projections =================
            with tc.tile_pool(name="xtp", bufs=1) as xtp:
                xt = [xtp.tile([128, T], F32R, tag=f"xt{cb}", name=f"xt{cb}")
                      for cb in range(NCB)]
                # chunked loads so chunk-0 compute starts after 2MB, not 8MB
                for tch in range(T // 512):
                    for cb in range(NCB):
                        csl = (slice(None), slice(tch * 512, (tch + 1) * 512))
                        nc.sync.dma_start(
                            xt[cb][csl],
                            xt_d[cb * 128:(cb + 1) * 128, tch * 512:(tch + 1) * 512].bitcast(F32R))

                proj_jobs = [(wsqk, None, None)]
                for h in range(2):
                    proj_jobs.append((wql, h, qtl[h]))
                    proj_jobs.append((wkl, h, ktl[h]))
                for tch in range(T // 512):
                    for w, h, dst in proj_jobs:
                        ps = bigps.tile([128, 1024], F32, tag="bigps", name="bigps")
                        for cb in range(NCB):
                            lhsT = w[:, cb, :] if h is None else w[:, cb, h * 128:(h + 1) * 128]
                            nc.tensor.matmul(
                                ps[:, 0:512], lhsT, xt[cb][:, tch * 512:(tch + 1) * 512],
                                start=(cb == 0), stop=(cb == NCB - 1),
                            )
                        sl = (slice(None), slice(tch * 512, (tch + 1) * 512))
                        # scalar engine is idle during the projection phase
                        if dst is None:
                            nc.scalar.copy(qts[sl], ps[0:64, 0:512])
                            nc.scalar.copy(kts[sl], ps[64:128, 0:512])
                        else:
                            nc.scalar.copy(dst[sl], ps[:, 0:512])
                    for tb in range(4 * tch, 4 * tch + 4):
                        ps = bigps.tile([128, 1024], F32, tag="bigps", name="bigps")
                        for cb in range(NCB):
                            nc.tensor.matmul(
                                ps[:, 0:256], xt[cb][:, tb * 128:(tb + 1) * 128], wv[:, cb, :],
                                start=(cb == 0), stop=(cb == NCB - 1),
                            )
                        for i in range(4):
                            nc.scalar.copy(
                                vt[i][:, tb * VW: tb * VW + HD], ps[:, i * 64:(i + 1) * 64]
                            )

            # ============ stage B: attention + output projection ============
            with (
                tc.tile_pool(name="attnc", bufs=1) as attnc,
                tc.tile_pool(name="ptp", bufs=4) as ptp,
                tc.tile_pool(name="ytp", bufs=2) as ytp,
                tc.tile_pool(name="obp", bufs=3) as obp,
                tc.tile_pool(name="smallp", bufs=2) as smallp,
            ):
                wp0 = attnc.tile([128, C], F32R, tag="wp0", name="wp0")
                nc.sync.dma_start(wp0[:], wp_d[0:128, :].bitcast(F32R))
                wp1 = attnc.tile([128, C], F32R, tag="wp1", name="wp1")
                nc.sync.dma_start(wp1[:], wp_d[128:256, :].bitcast(F32R))
                band_s = attnc.tile([128, WIN_S + 896], F32R, tag="band_s", name="band_s")
                nc.sync.dma_start(band_s[:], bs_d[:, :].bitcast(F32R))
                band_l = attnc.tile([128, WIN_L + 896], F32R, tag="band_l", name="band_l")
                nc.sync.dma_start(band_l[:], bl_d[:, :].bitcast(F32R))
                onesr = attnc.tile([128, 64], F32, tag="onesr", name="onesr")
                nc.sync.dma_start(onesr[:], ones_d[:, :])
                # ones column of each v block (strided view [:, 64::65])
                for i in range(4):
                    v3 = vt[i][:, :].rearrange("p (nt vw) -> p nt vw", vw=VW)
                    nc.sync.dma_start(v3[:, :, HD], ones_d[:, 0:NT].bitcast(F32R))

                for qg in range(NG):
                    q0 = qg * 512
                    yts = [ytp.tile([128, 512], F32R, tag=f"yts{i}", name=f"yts{i}")
                           for i in range(2)]

                    heads = []
                    for h in range(2):  # short heads
                        heads.append((
                            lambda kb, h=h: kts[32 * h: 32 * h + 32, kb * 128:(kb + 1) * 128],
                            qts[32 * h: 32 * h + 32, q0: q0 + 512],
                            vt[h], WIN_S, scale_s, band_s, yts[0], 64 * h,
                        ))
                    for h in range(2):  # long heads
                        heads.append((
                            lambda kb, h=h: ktl[h][:, kb * 128:(kb + 1) * 128],
                            qtl[h][:, q0: q0 + 512],
                            vt[2 + h], WIN_L, scale_l, band_l, yts[1], 64 * h,
                        ))

                    s4 = smallp.tile([97, 512], F32, tag="s4", name="s4")
                    r4 = smallp.tile([97, 512], F32, tag="r4", name="r4")
                    yhs = []
                    for hi, (kt_ap, qt_ap, v_tile, win, scale, band, dest, poff) in enumerate(heads):
                        kb_lo = max(0, q0 - win) // 128
                        kb_hi = (q0 + 384) // 128
                        kbs = list(range(kb_lo, kb_hi + 1))
                        yh = yhps.tile([VW, 512], F32, tag="yh", name="yh")
                        # process key blocks in pairs sharing a 2-bank psum
                        # tile and a single exp instruction
                        pt_slices = []
                        for j in range(0, len(kbs), 2):
                            pair = kbs[j: j + 2]
                            wdt = 512 * len(pair)
                            st = bigps.tile([128, 1024], F32, tag="bigps", name="bigps")
                            for jj, kb in enumerate(pair):
                                nc.tensor.matmul(
                                    st[:, jj * 512:(jj + 1) * 512], kt_ap(kb), qt_ap,
                                    start=True, stop=True)
                            pt = ptp.tile([128, 1024], F32R, tag="pt", name="pt")
                            nc.scalar.activation(
                                pt[:, 0:wdt], st[:, 0:wdt],
                                mybir.ActivationFunctionType.Exp, scale=scale)
                            for jj, kb in enumerate(pair):
                                delta = kb * 128 - q0
                                psl = (slice(None), slice(jj * 512, (jj + 1) * 512))
                                if not (512 - win <= delta <= -128):
                                    off = 384 - delta
                                    eng = nc.vector if (kb + qg) % 2 == 0 else nc.gpsimd
                                    eng.tensor_tensor(out=pt[psl], in0=pt[psl],
                                                      in1=band[:, off: off + 512],
                                                      op=mybir.AluOpType.mult)
                                pt_slices.append((kb, pt, psl))
                        for i, (kb, pt, psl) in enumerate(pt_slices):
                            nc.tensor.matmul(
                                yh[:], v_tile[:, kb * VW:(kb + 1) * VW], pt[psl],
                                start=(i == 0), stop=(i == len(pt_slices) - 1),
                            )
                        nc.vector.tensor_copy(s4[32 * hi: 32 * hi + 1, :], yh[HD: HD + 1, :])
                        yv = smallp.tile([64, 512], F32, tag="yv", name="yv", bufs=4)
                        nc.vector.tensor_copy(yv[:], yh[0:HD, :])
                        yhs.append((yv, dest, poff))
                    nc.vector.reciprocal(r4[:], s4[:])
                    # matmul operand base partitions are limited to {0, 32, 64}:
                    # relocate head 3's reciprocal row to partition 0
                    r3 = smallp.tile([1, 512], F32, tag="r3", name="r3")
                    nc.vector.tensor_copy(r3[:], r4[96:97, :])
                    for hi, (yv, dest, poff) in enumerate(yhs):
                        rb = rbps.tile([64, 512], F32, tag="rb", name="rb")
                        rsrc = r3[0:1, :] if hi == 3 else r4[32 * hi: 32 * hi + 1, :]
                        osrc = onesr[0:1, :] if hi == 3 else onesr[32 * hi: 32 * hi + 1, :]
                        nc.tensor.matmul(rb[:], osrc, rsrc, start=True, stop=True)
                        with nc.allow_low_precision(reason="f32r rounding of attn out"):
                            nc.vector.tensor_mul(dest[poff: poff + 64, :], yv[:], rb[:])

                    for sub in range(4):
                        qs = q0 + sub * 128
                        ssl = (slice(None), slice(sub * 128, (sub + 1) * 128))
                        for nh in range(2):
                            po = bigps.tile([128, 1024], F32, tag="bigps", name="bigps")
                            nc.tensor.matmul(po[:, 0:512], yts[0][ssl], wp0[:, nh * 512:(nh + 1) * 512],
                                             start=True, stop=False)
                            nc.tensor.matmul(po[:, 0:512], yts[1][ssl], wp1[:, nh * 512:(nh + 1) * 512],
                                             start=False, stop=True)
                            ob = obp.tile([128, 512], F32, tag="ob", name="ob")
                            nc.vector.tensor_copy(ob[:], po[:, 0:512])
                            nc.sync.dma_start(out_d[qs: qs + 128, nh * 512:(nh + 1) * 512], ob[:])

    return nc


_PROGRAM = None


def _get_program() -> bass.Bass:
    global _PROGRAM
    if _PROGRAM is None:
        _PROGRAM = _build_program()
        _split_waits(_PROGRAM)
    return _PROGRAM


def _band_image(win: int) -> np.ndarray:
    """[128, win+896] 0/1 image: B[r, u] = 1 iff (u - 384 - r) in [0, win)."""
    u = np.arange(win + 896)[None, :]
    r = np.arange(128)[:, None]
    d = u - 384 - r
    return ((d >= 0) & (d < win)).astype(np.float32)


def make_in_maps(x, Wqk_short, Wv_short, Wqk_long, Wv_long, Wproj):
    """Host-side sharding: per-core input dict for core c = 4*b + g."""
    x = np.ascontiguousarray(np.asarray(x, dtype=np.float32))
    Wqk_short = np.asarray(Wqk_short, dtype=np.float32)
    Wv_short = np.asarray(Wv_short, dtype=np.float32)
    Wqk_long = np.asarray(Wqk_long, dtype=np.float32)
    Wv_long = np.asarray(Wv_long, dtype=np.float32)
    Wproj = np.asarray(Wproj, dtype=np.float32)
    assert x.shape == (B, T, C)

    xts = [np.ascontiguousarray(x[b].T) for b in range(B)]
    band_s = _band_image(WIN_S)
    band_l = _band_image(WIN_L)
    ones = np.ones((128, 64), dtype=np.float32)
    in_maps = []
    for c in range(N_CORES):
        b, g = divmod(c, 4)
        wsqk = np.ascontiguousarray(np.concatenate(
            [Wqk_short[:, g * 64:(g + 1) * 64],
             Wqk_short[:, 256 + g * 64: 256 + (g + 1) * 64]], axis=1))
        wql = np.ascontiguousarray(Wqk_long[:, g * 256:(g + 1) * 256])
        wkl = np.ascontiguousarray(Wqk_long[:, 1024 + g * 256: 1024 + (g + 1) * 256])
        wv = np.ascontiguousarray(np.concatenate(
            [Wv_short[:, g * 128:(g + 1) * 128],
             Wv_long[:, g * 128:(g + 1) * 128]], axis=1))
        wp = np.ascontiguousarray(np.concatenate(
            [Wproj[g * 128:(g + 1) * 128, :],
             Wproj[512 + g * 128: 512 + (g + 1) * 128, :]], axis=0))
        in_maps.append({
            "xt": xts[b], "wsqk": wsqk, "wql": wql, "wkl": wkl, "wv": wv, "wp": wp,
            "band_s": band_s, "band_l": band_l, "ones": ones,
        })
    return in_maps


def gather(results) -> np.ndarray:
    out = np.empty((B, T, C), dtype=np.float32)
    for b in range(B):
        acc = np.zeros((T, C), dtype=np.float64)
        for g in range(4):
            acc += results[4 * b + g]["out"]
        out[b] = acc.astype(np.float32)
    return out


def kernel(x, Wqk_short, Wv_short, Wqk_long, Wv_long, Wproj, **run_kwargs):
    nc = _get_program()
    in_maps = make_in_maps(x, Wqk_short, Wv_short, Wqk_long, Wv_long, Wproj)
    res = run_bass_kernel_spmd(nc, in_maps, core_ids=list(range(N_CORES)), **run_kwargs)
    out = gather(res.results)
    if run_kwargs:
        kernel.last_results = res
    return out



# revision 16
# speedup vs baseline: 1.5239x; 1.5239x over previous
"""Trainium2 Bass kernel for a two-window sparse causal self-attention block.

Model (B=2, T=2048, C=1024):
  - 8 "short" heads: d_qk=32,  window 256
  - 8 "long"  heads: d_qk=128, window 1024
  - value/output head dim 64, output projection C x C.

Sharding (8 cores): data-parallel over batch (2) x head-parallel over head
groups (4). Core c = 4*b + g handles batch b and heads {2g, 2g+1} of both the
short and long sets. Each core computes its 4 heads' attention plus the
corresponding 256 rows of Wproj, producing a partial [T, C] output (bf16);
the host sums the 4 partials per batch element in fp32.

V2 design notes (vs the fp32r baseline):
  - bf16 operands everywhere on the PE (fp32 PSUM accumulation): halves HBM
    traffic for x / out, enables FWL fast weight loads, and doubles DVE
    throughput for the mask multiplies. Correctness gate is 2e-2.
  - windowed score/AV matmuls: for key-tile kb only the 128-rounded valid
    query window [max(q0,128kb), min(q0+512, 128kb+win+128)) is computed,
    cutting streamed PE columns ~50% (short) / ~25% (long) and cutting exp
    and mask element counts the same way. The band-image mask zeroes the
    rounding slack.
  - score windows are packed into [128, <=1024] PSUM group tiles so one
    scalar-engine exp serves many windows (matmuls split at the 512-col PSUM
    bank boundary).
  - engine assignment: exp -> Scalar; band masks -> GpSimd; copies /
    reciprocal / normalization multiplies -> Vector; projections' PSUM
    drains -> Scalar (idle in stage A).
  - softmax normalization: ones-column in v accumulates query sums during
    AV; reciprocal_approx_fast (~18 bits) into r4 rows {0,32,64,96}; a K=33
    indicator matmul broadcasts two heads' reciprocal rows into a [128,512]
    tile per yts half. The normalization + output projection of query group
    g are deferred into group g+1's score phase so the PE never waits on the
    DVE chain.
  - x is DMA'd in 512-token chunks issued before the large weights so the
    first projection matmul starts early.
"""

import math

import numpy as np
from ml_dtypes import bfloat16

import concourse.bass as bass
import concourse.mybir as mybir
import concourse.tile as tile
from concourse.bass_utils import run_bass_kernel_spmd

F32 = mybir.dt.float32
BF16 = mybir.dt.bfloat16

B, T, C = 2, 2048, 1024
HS, DS = 8, 32
HL, DL = 8, 128
HD = 64
WIN_S, WIN_L = 256, 1024
NT = T // 128    # 16 t-blocks
NCB = C // 128   # 8 c-blocks
NG = T // 512    # 4 query groups
VW = HD + 1      # v columns + ones column for softmax sums
N_CORES = 8


def _split_waits(nc: bass.Bass) -> int:
    """Walrus in this env accepts at most 1 sync wait per instruction.
    Hoist extra waits onto same-engine InstNoOp instructions placed just
    before the owning instruction (same-engine program order preserves the
    blocking semantics)."""
    import bass_rust

    n_added = 0
    for f in nc.m.functions:
        for bb in f.blocks:
            insts = bb.instructions
            if not any(inst.sync_info and len(inst.sync_info.on_wait) > 1
                       for inst in insts):
                continue
            new = []
            for inst in insts:
                si = inst.sync_info
                waits = list(si.on_wait) if si else []
                if len(waits) > 1:
                    for i, w in enumerate(waits[:-1]):
                        nop = mybir.InstNoOp(
                            name=f"{inst.name}_hw{i}",
                            sync_info=bass_rust.SyncInfo(on_wait=[w], on_update=[]),
                            bass_nofuse=True,
                            engine=inst.engine,
                        )
                        new.append(nop)
                        n_added += 1
                    inst.sync_info = bass_rust.SyncInfo(
                        on_wait=waits[-1:], on_update=list(si.on_update))
                new.append(inst)
            bb.instructions = new
    return n_added


def _patch_tile_drain():
    """This walrus build rejects >1 sync wait on the TileContext tail drain
    ("Too many sync wait commands"). Re-emit the drain's waits as individual
    wait_ge instructions on the sync engine."""
    import bass_rust
    from concourse.tile import ScopedClock, TileContext

    def _drain_and_barrier(self, tick_clock, wait_clock):
        nc = self.nc
        drain_inst = nc.sync.drain()
        wait_clock.add_sem_waits(
            drain_inst.ins, ScopedClock({None: tick_clock.global_clock})
        )
        si = drain_inst.ins.sync_info
        waits = list(si.on_wait) if si is not None else []
        if len(waits) > 1:
            drain_inst.ins.sync_info = bass_rust.SyncInfo(on_wait=[], on_update=[])
            sems = {h.name: h for h in self.sems.allocated().values()}
            for w in waits:
                nc.sync.wait_ge(sems[w.ant_name], w.wait_value)
        nc.all_engine_barrier()
        popped = nc._tile_sem_poison_stack.pop()
        assert popped is self._sem_poison
        nc.clear_and_free_semaphores(list(self.sems.allocated().values()))
        nc.all_engine_barrier()

    TileContext._drain_and_barrier = _drain_and_barrier


_patch_tile_drain()


def _windows(q0: int, win: int):
    """Valid query windows per key-tile for queries [q0, q0+512).
    Returns [(kb, qlo, w, masked)]; qlo/w are 128-aligned and the window is
    the 128-rounded exact valid query range of key-tile kb."""
    kb_lo = max(0, q0 - win) // 128
    kb_hi = (q0 + 384) // 128
    out = []
    for kb in range(kb_lo, kb_hi + 1):
        qlo = max(q0, kb * 128)
        qhi = min(q0 + 512, kb * 128 + win + 128)
        w = qhi - qlo
        if w <= 0:
            continue
        masked = not (qlo >= kb * 128 + 128 and qlo + w <= kb * 128 + win)
        out.append((kb, qlo, w, masked))
    return out


def _groups(q0: int, win: int):
    """Pack windows into score-group tiles of <=1024 psum columns, masked
    windows first within each group so one band multiply covers the group's
    masked prefix. Returns [([(kb, qlo, w, masked, off), ...], mcols), ...]."""
    groups, cur, off = [], [], 0
    for kb, qlo, w, masked in _windows(q0, win):
        if off + w > 1024:
            groups.append(cur)
            cur, off = [], 0
        cur.append((kb, qlo, w, masked))
        off += w
    if cur:
        groups.append(cur)
    out = []
    for grp in groups:
        grp = sorted(grp, key=lambda t: not t[3])  # masked first, stable
        off, mcols, placed = 0, 0, []
        for kb, qlo, w, masked in grp:
            placed.append((kb, qlo, w, masked, off))
            if masked:
                mcols += w
            off += w
        out.append((placed, mcols))
    return out


def _gmask_layout():
    """Column layout of the concatenated masked-window band images, shared
    by host (image build) and device (slice offsets).
    Returns ({(q0, win): [goff per group]}, total_cols)."""
    offs, total = {}, 0
    for qg in range(NG):
        for win in (WIN_S, WIN_L):
            lst = []
            for placed, mcols in _groups(qg * 512, win):
                lst.append(total)
                total += mcols
            offs[(qg * 512, win)] = lst
    return offs, total


_GOFFS, _GMASK_COLS = _gmask_layout()


def _build_program() -> bass.Bass:
    nc = bass.Bass()

    xt_d = nc.dram_tensor("xt", [C, T], BF16, kind="ExternalInput")
    wsqk_d = nc.dram_tensor("wsqk", [C, 128], BF16, kind="ExternalInput")
    wql_d = nc.dram_tensor("wql", [C, 256], BF16, kind="ExternalInput")
    wkl_d = nc.dram_tensor("wkl", [C, 256], BF16, kind="ExternalInput")
    wv_d = nc.dram_tensor("wv", [C, 256], BF16, kind="ExternalInput")
    wp_d = nc.dram_tensor("wp", [256, C], BF16, kind="ExternalInput")
    gm_d = nc.dram_tensor("gmask", [128, _GMASK_COLS], BF16, kind="ExternalInput")
    ind_d = nc.dram_tensor("ind2", [97, 128], F32, kind="ExternalInput")
    out_d = nc.dram_tensor("out", [T, C], BF16, kind="ExternalOutput")

    scale_s = 1.0 / math.sqrt(DS)
    scale_l = 1.0 / math.sqrt(DL)

    with nc.allow_low_precision(reason="bf16 attention pipeline, gate is 2e-2"), \
         tile.TileContext(nc) as tc:
        with (
            tc.tile_pool(name="const", bufs=1) as const,
            tc.tile_pool(name="qkp", bufs=1) as qkp,
            tc.tile_pool(name="vp", bufs=1) as vp,
            tc.tile_pool(name="bigps", bufs=2, space="PSUM") as bigps,
            tc.tile_pool(name="yhps", bufs=2, space="PSUM") as yhps,
            tc.tile_pool(name="pops", bufs=2, space="PSUM") as pops,
        ):
            # ---- projection outputs (persist across both stages) ----
            qts = qkp.tile([64, T], BF16, tag="qts", name="qts")
            kts = qkp.tile([64, T], BF16, tag="kts", name="kts")
            qtl = [qkp.tile([128, T], BF16, tag=f"qtl{h}", name=f"qtl{h}") for h in range(2)]
            ktl = [qkp.tile([128, T], BF16, tag=f"ktl{h}", name=f"ktl{h}") for h in range(2)]
            vt = [vp.tile([128, NT * VW], BF16, tag=f"vt{i}", name=f"vt{i}") for i in range(4)]

            # ================= stage A: projections =================
            with tc.tile_pool(name="xtp", bufs=1) as xtp:
                xt = [xtp.tile([128, T], BF16, tag=f"xt{cb}", name=f"xt{cb}")
                      for cb in range(NCB)]
                wsqk = const.tile([128, NCB, 128], BF16, tag="wsqk", name="wsqk")
                wql = const.tile([128, NCB, 256], BF16, tag="wql", name="wql")
                wkl = const.tile([128, NCB, 256], BF16, tag="wkl", name="wkl")
                wv = const.tile([128, NCB, 256], BF16, tag="wv", name="wv")
                # first chunk of x before the big weights so compute starts
                # early; remaining chunks stream behind.
                for tch in range(T // 512):
                    for cb in range(NCB):
                        csl = (slice(None), slice(tch * 512, (tch + 1) * 512))
                        nc.sync.dma_start(
                            xt[cb][csl],
                            xt_d[cb * 128:(cb + 1) * 128, tch * 512:(tch + 1) * 512])
                    if tch == 0:
                        nc.sync.dma_start(wsqk[:], wsqk_d[:, :].rearrange("(cb p) d -> p cb d", p=128))
                        nc.sync.dma_start(wql[:], wql_d[:, :].rearrange("(cb p) d -> p cb d", p=128))
                        nc.sync.dma_start(wkl[:], wkl_d[:, :].rearrange("(cb p) d -> p cb d", p=128))
                        nc.sync.dma_start(wv[:], wv_d[:, :].rearrange("(cb p) d -> p cb d", p=128))

                # ones columns of the v tiles (strided view [:, 64::65])
                for i in range(4):
                    v3 = vt[i][:, :].rearrange("p (nt vw) -> p nt vw", vw=VW)
                    nc.gpsimd.memset(v3[:, :, HD], 1.0)

                proj_jobs = [(wsqk, None, None)]
                for h in range(2):
                    proj_jobs.append((wql, h, qtl[h]))
                    proj_jobs.append((wkl, h, ktl[h]))
                for tch in range(T // 512):
                    for w, h, dst in proj_jobs:
                        ps = bigps.tile([128, 1024], F32, tag="bigps", name="bigps")
                        for cb in range(NCB):
                            lhsT = w[:, cb, :] if h is None else w[:, cb, h * 128:(h + 1) * 128]
                            nc.tensor.matmul(
                                ps[:, 0:512], lhsT, xt[cb][:, tch * 512:(tch + 1) * 512],
                                start=(cb == 0), stop=(cb == NCB - 1),
                            )
                        sl = (slice(None), slice(tch * 512, (tch + 1) * 512))
                        # scalar engine is idle during the projection phase
                        if dst is None:
                            nc.scalar.copy(qts[sl], ps[0:64, 0:512])
                            nc.scalar.copy(kts[sl], ps[64:128, 0:512])
                        else:
                            nc.scalar.copy(dst[sl], ps[:, 0:512])
                    for tb in range(4 * tch, 4 * tch + 4):
                        ps = bigps.tile([128, 1024], F32, tag="bigps", name="bigps")
                        for cb in range(NCB):
                            nc.tensor.matmul(
                                ps[:, 0:256], xt[cb][:, tb * 128:(tb + 1) * 128], wv[:, cb, :],
                                start=(cb == 0), stop=(cb == NCB - 1),
                            )
                        for i in range(4):
                            nc.scalar.copy(
                                vt[i][:, tb * VW: tb * VW + HD], ps[:, i * 64:(i + 1) * 64]
                            )

            # ============ stage B: attention + output projection ============
            with (
                tc.tile_pool(name="attnc", bufs=1) as attnc,
                tc.tile_pool(name="ptp", bufs=16) as ptp,
                tc.tile_pool(name="ytp", bufs=2) as ytp,
                tc.tile_pool(name="obp", bufs=3) as obp,
                tc.tile_pool(name="smallp", bufs=2) as smallp,
            ):
                wp0 = attnc.tile([128, C], BF16, tag="wp0", name="wp0")
                nc.sync.dma_start(wp0[:], wp_d[0:128, :])
                wp1 = attnc.tile([128, C], BF16, tag="wp1", name="wp1")
                nc.sync.dma_start(wp1[:], wp_d[128:256, :])
                gmask = attnc.tile([128, _GMASK_COLS], BF16, tag="gmask", name="gmask")
                nc.sync.dma_start(gmask[:], gm_d[:, :])
                ind2 = attnc.tile([97, 128], F32, tag="ind2", name="ind2")
                nc.sync.dma_start(ind2[:], ind_d[:, :])
                # rsum rows {0,32,64,96} collect per-head softmax sums; r4
                # holds their reciprocals. Other rows feed the K=33 indicator
                # matmul as zero-weight operands and must be finite.
                rsum = attnc.tile([97, 512], F32, tag="rsum", name="rsum")
                nc.vector.memset(rsum[:], 1.0)
                r4 = attnc.tile([97, 512], F32, tag="r4", name="r4")
                nc.vector.memset(r4[:], 1.0)

                def head_cfgs():
                    cfgs = []
                    for h in range(2):   # short heads
                        cfgs.append(dict(
                            kt=lambda kb, h=h: kts[32 * h: 32 * h + 32, kb * 128:(kb + 1) * 128],
                            qt=lambda qlo, w, h=h: qts[32 * h: 32 * h + 32, qlo: qlo + w],
                            v=vt[h], win=WIN_S, scale=scale_s,
                        ))
                    for h in range(2):   # long heads
                        cfgs.append(dict(
                            kt=lambda kb, h=h: ktl[h][:, kb * 128:(kb + 1) * 128],
                            qt=lambda qlo, w, h=h: qtl[h][:, qlo: qlo + w],
                            v=vt[2 + h], win=WIN_L, scale=scale_l,
                        ))
                    return cfgs

                cfgs = head_cfgs()

                def emit_scores(qg, hi):
                    """Score matmuls + exp + group band mask for one head.
                    Returns the pt windows [(kb, qlo, w, pt_tile, off)]."""
                    cfg = cfgs[hi]
                    q0 = qg * 512
                    pt_windows = []
                    goffs = _GOFFS[(q0, cfg["win"])]
                    for gi, (placed, mcols) in enumerate(_groups(q0, cfg["win"])):
                        used = placed[-1][4] + placed[-1][2]
                        st = bigps.tile([128, 1024], F32, tag="bigps", name="bigps")
                        for kb, qlo, w, masked, off in placed:
                            # split at the 512-col psum bank boundary
                            cuts = [0]
                            if off < 512 < off + w:
                                cuts.append(512 - off)
                            cuts.append(w)
                            for a, b in zip(cuts, cuts[1:]):
                                nc.tensor.matmul(
                                    st[:, off + a: off + b], cfg["kt"](kb),
                                    cfg["qt"](qlo + a, b - a),
                                    start=True, stop=True)
                        pt = ptp.tile([128, 1024], BF16, tag="pt", name="pt")
                        nc.scalar.activation(
                            pt[:, 0:used], st[:, 0:used],
                            mybir.ActivationFunctionType.Exp, scale=cfg["scale"])
                        if mcols:
                            # one multiply over the group's masked prefix;
                            # short heads' masks run on GpSimd, long on DVE
                            eng = nc.gpsimd if cfg["win"] == WIN_S else nc.vector
                            eng.tensor_tensor(
                                out=pt[:, 0:mcols], in0=pt[:, 0:mcols],
                                in1=gmask[:, goffs[gi]: goffs[gi] + mcols],
                                op=mybir.AluOpType.mult)
                        for kb, qlo, w, masked, off in placed:
                            pt_windows.append((kb, qlo, w, pt, off))
                    return pt_windows

                def emit_av(qg, hi, pt_windows, yv2):
                    """AV accumulation + sums-row copy + stacked yv copy."""
                    cfg = cfgs[hi]
                    q0 = qg * 512
                    yh = yhps.tile([VW, 512], F32, tag="yh", name="yh")
                    n = len(pt_windows)
                    for i, (kb, qlo, w, pt, off) in enumerate(pt_windows):
                        nc.tensor.matmul(
                            yh[:, qlo - q0: qlo - q0 + w],
                            cfg["v"][:, kb * VW:(kb + 1) * VW],
                            pt[:, off: off + w],
                            start=(i == 0), stop=(i == n - 1),
                            skip_group_check=True)
                    nc.scalar.copy(rsum[32 * hi: 32 * hi + 1, :], yh[HD: HD + 1, :])
                    poff = 64 * (hi % 2)
                    nc.vector.tensor_copy(yv2[hi // 2][poff: poff + 64, :], yh[0:HD, :])

                def emit_norm(yts, yv2):
                    """Broadcast reciprocals via K=33 indicator matmuls and
                    normalize into the bf16 yts tiles."""
                    for half in range(2):
                        rb = pops.tile([128, 512], F32, tag="pops", name="pops")
                        nc.tensor.matmul(rb[:], ind2[64 * half: 64 * half + 33, :],
                                         r4[64 * half: 64 * half + 33, :],
                                         start=True, stop=True)
                        rbs = smallp.tile([128, 512], BF16, tag="rbs", name="rbs")
                        nc.vector.tensor_copy(rbs[:], rb[:])
                        nc.vector.tensor_mul(yts[half][:], yv2[half][:], rbs[:])

                def emit_outproj(qg, yts):
                    q0 = qg * 512
                    for sub in range(4):
                        qs = q0 + sub * 128
                        ssl = (slice(None), slice(sub * 128, (sub + 1) * 128))
                        for nh in range(2):
                            po = pops.tile([128, 512], F32, tag="pops", name="pops")
                            nc.tensor.matmul(po[:], yts[0][ssl], wp0[:, nh * 512:(nh + 1) * 512],
                                             start=True, stop=False)
                            nc.tensor.matmul(po[:], yts[1][ssl], wp1[:, nh * 512:(nh + 1) * 512],
                                             start=False, stop=True)
                            ob = obp.tile([128, 512], BF16, tag="ob", name="ob")
                            if sub * 2 + nh in (3, 7):
                                nc.scalar.copy(ob[:], po[:])
                            else:
                                nc.vector.tensor_copy(ob[:], po[:])
                            nc.sync.dma_start(out_d[qs: qs + 128, nh * 512:(nh + 1) * 512], ob[:])

                pending = None
                for qg in range(NG):
                    yts = [ytp.tile([128, 512], BF16, tag=f"yts{i}", name=f"yts{i}")
                           for i in range(2)]
                    yv2 = [smallp.tile([128, 512], BF16, tag=f"yv2{i}", name=f"yv2{i}")
                           for i in range(2)]

                    all_pt = [None] * 4
                    all_pt[0] = emit_scores(qg, 0)
                    # deferred normalization of qg-1: its DVE chain overlaps
                    # the remaining score phase
                    if pending is not None:
                        p_yts, p_yv2, p_qg = pending
                        emit_norm(p_yts, p_yv2)
                    all_pt[1] = emit_scores(qg, 1)
                    all_pt[2] = emit_scores(qg, 2)
                    # deferred output projection of qg-1
                    if pending is not None:
                        emit_outproj(p_qg, p_yts)
                        pending = None
                    all_pt[3] = emit_scores(qg, 3)

                    for hi in range(4):
                        emit_av(qg, hi, all_pt[hi], yv2)
                    nc.vector.reciprocal(r4[:], rsum[:])

                    pending = (yts, yv2, qg)

                p_yts, p_yv2, p_qg = pending
                emit_norm(p_yts, p_yv2)
                emit_outproj(p_qg, p_yts)

    return nc


_PROGRAM = None


def _get_program() -> bass.Bass:
    global _PROGRAM
    if _PROGRAM is None:
        _PROGRAM = _build_program()
        _split_waits(_PROGRAM)
    return _PROGRAM


def _gmask_image() -> np.ndarray:
    """[128, _GMASK_COLS] 0/1 image: the masked-prefix windows of every score
    group, concatenated in _gmask_layout order. Window (kb, qlo, w) column u
    covers query qlo+u against key 128*kb+r."""
    img = np.zeros((128, _GMASK_COLS), dtype=np.float32)
    r = np.arange(128)[:, None]
    col = 0
    for qg in range(NG):
        for win in (WIN_S, WIN_L):
            for placed, mcols in _groups(qg * 512, win):
                for kb, qlo, w, masked, off in placed:
                    if not masked:
                        continue
                    u = np.arange(w)[None, :]
                    d = (qlo + u) - (kb * 128 + r)
                    img[:, col: col + w] = (d >= 0) & (d < win)
                    col += w
    assert col == _GMASK_COLS
    return img


def make_in_maps(x, Wqk_short, Wv_short, Wqk_long, Wv_long, Wproj):
    """Host-side sharding: per-core input dict for core c = 4*b + g."""
    x = np.asarray(x, dtype=np.float32)
    Wqk_short = np.asarray(Wqk_short, dtype=np.float32)
    Wv_short = np.asarray(Wv_short, dtype=np.float32)
    Wqk_long = np.asarray(Wqk_long, dtype=np.float32)
    Wv_long = np.asarray(Wv_long, dtype=np.float32)
    Wproj = np.asarray(Wproj, dtype=np.float32)
    assert x.shape == (B, T, C)

    bf = bfloat16
    xts = [np.ascontiguousarray(x[b].T).astype(bf) for b in range(B)]
    gmask = _gmask_image().astype(bf)
    ind2 = np.zeros((97, 128), dtype=np.float32)
    ind2[0, 0:64] = 1.0
    ind2[32, 64:128] = 1.0
    ind2[64, 0:64] = 1.0
    ind2[96, 64:128] = 1.0
    in_maps = []
    for c in range(N_CORES):
        b, g = divmod(c, 4)
        wsqk = np.ascontiguousarray(np.concatenate(
            [Wqk_short[:, g * 64:(g + 1) * 64],
             Wqk_short[:, 256 + g * 64: 256 + (g + 1) * 64]], axis=1)).astype(bf)
        wql = np.ascontiguousarray(Wqk_long[:, g * 256:(g + 1) * 256]).astype(bf)
        wkl = np.ascontiguousarray(Wqk_long[:, 1024 + g * 256: 1024 + (g + 1) * 256]).astype(bf)
        wv = np.ascontiguousarray(np.concatenate(
            [Wv_short[:, g * 128:(g + 1) * 128],
             Wv_long[:, g * 128:(g + 1) * 128]], axis=1)).astype(bf)
        wp = np.ascontiguousarray(np.concatenate(
            [Wproj[g * 128:(g + 1) * 128, :],
             Wproj[512 + g * 128: 512 + (g + 1) * 128, :]], axis=0)).astype(bf)
        in_maps.append({
            "xt": xts[b], "wsqk": wsqk, "wql": wql, "wkl": wkl, "wv": wv, "wp": wp,
            "gmask": gmask, "ind2": ind2,
        })
    return in_maps


def gather(results) -> np.ndarray:
    out = np.empty((B, T, C), dtype=np.float32)
    for b in range(B):
        acc = np.zeros((T, C), dtype=np.float32)
        for g in range(4):
            acc += results[4 * b + g]["out"].astype(np.float32)
        out[b] = acc
    return out


def kernel(x, Wqk_short, Wv_short, Wqk_long, Wv_long, Wproj, **run_kwargs):
    nc = _get_program()
    in_maps = make_in_maps(x, Wqk_short, Wv_short, Wqk_long, Wv_long, Wproj)
    res = run_bass_kernel_spmd(nc, in_maps, core_ids=list(range(N_CORES)), **run_kwargs)
    out = gather(res.results)
    if run_kwargs:
        kernel.last_results = res
    return out
